# revision 1
# baseline (speedup 1.0000x reference)
"""Trainium2 Bass kernel for nn_CNN_pre_LSTM (dense_cnn).

Reference computation per sample (L=24):
    h = relu(conv1d(x, w11, b11))    # 1 -> 8 ch, k=3, same pad
    h = relu(conv1d(h, w12, b12))    # 8 -> 8
    h = maxpool2(h)                  # L 24 -> 12
    h = relu(conv1d(h, w21, b21))    # 8 -> 16
    h = relu(conv1d(h, w22, b22))    # 16 -> 16
    h = maxpool2(h)                  # L 12 -> 6
    y = h.reshape(96) @ Wl.T + bl    # 96 -> 24

Mapping: pure data parallel over the fused (S*B) batch across 8 cores;
16384 samples per core. On chip, activations live as [feature, batch_tile]
(features on SBUF partitions, batch on the free dim); each conv layer is
ONE dense banded matmul per 128-row output block (host-built matrices,
l-major/c-minor feature order, halo-overlapped l-halves so there is no
PSUM accumulation anywhere):

  - conv11 blocks evacuate PSUM via ACT (relu + per-partition bias);
    conv21 blocks via DVE tensor_scalar ((x+b) max 0) to balance engines.
  - pooled layers (conv12, conv22) emit parity-grouped blocks (even l at
    rows 0:48, odd l at rows 64:112 of one tensor). After a bias+relu
    evacuation, a small SBUF->SBUF DMA aligns the odd block's partitions
    and DVE tensor_max pools (all compute-engine operands must live on
    identical partition ranges; only DMA can move data across partitions).
  - every DMA is issued by the engine that produces its source (moves on
    ACT, which just computed the source; output store on GPSIMD; input
    prefetch alone on the sync queue) so no sequencer's program order
    serializes consecutive tiles.

The PE on this part runs at a fixed 1.2 GHz (the HAM clock gate never
opens even under 6us+ of continuous matmuls), so matmul cost is N/1.2GHz
per instruction and the matmul-instruction count (18 per 1024-sample
tile) is the kernel's hard floor.

The input is pre-transposed/chunked on the host to [n_tiles, 24, NT] per
core (DRAM partition strides must stay <= 32KB; 64KB strides crash the
device), and the output is produced as [n_tiles, 24, NT] fp32 and
reassembled on the host. All weights/biases ship as two packed blobs
(one DMA each at kernel start).
"""

import numpy as np

import concourse.bass as bass
import concourse.tile as tile
import concourse.mybir as mybir
from concourse import bacc
from concourse.bass_utils import run_bass_kernel_spmd

# ---------------------------------------------------------------- config
N_CORES = 8
S, B, L = 512, 256, 24
SB = S * B
CORE_N = SB // N_CORES  # 16384

# compute dtype for matmul operands / intermediate activations:
#   "fp16"  : float16 operands, fp32 PSUM accumulate, NT=1024
#   "fp32r" : fp32 bits, PE in float32r mode, NT=512
#   "fp32"  : exact fp32 (PE 4x slower), NT=512
COMPUTE = "fp16"


def _cfg(compute):
    if compute == "fp16":
        return dict(dt=mybir.dt.float16, np_dt=np.float16, nt=1024, mm_cast=None)
    if compute == "fp32r":
        return dict(
            dt=mybir.dt.float32, np_dt=np.float32, nt=512, mm_cast=mybir.dt.float32r
        )
    if compute == "fp32":
        return dict(dt=mybir.dt.float32, np_dt=np.float32, nt=512, mm_cast=None)
    raise ValueError(compute)


# ------------------------------------------------- host weight transforms
#
# Feature row orderings (all l-major, c-minor):
#   h1 block A: rows (l, c)  l in [0,13), c in [0,8)   -> 104 rows
#   h1 block B: rows (l, c)  l in [11,24)              -> 104 rows
#   conv12 out (parity): rows par*64 + lp*8 + c        -> 112 rows used
#   pooled h2:  rows [lp 0..5 x8ch | 16 pad | lp 6..11 x8ch] = 112
#   h3 block A: rows (l, c16) l in [0,7)               -> 112 rows
#   h3 block B: rows (l-5, c16) l in [5,12)            -> 112 rows
#   conv22 out (parity): rows par*64 + lp*16 + c       -> 112 rows used
#   pooled h4:  rows [lp 0..2 x16ch | 16 pad | lp 3..5 x16ch] = 112
#   out: rows j in [0,24)

def _band_first(w, l_ins, l_outs, cin, cout):
    """Dense banded matrix [len(l_ins)*cin, len(l_outs)*cout] for a k=3
    'same' conv, rows (l_in, ci) l-major, cols (l_out, co) l-major."""
    K = len(l_ins) * cin
    M = len(l_outs) * cout
    W = np.zeros((K, M), np.float32)
    for ki, li in enumerate(l_ins):
        for ci in range(cin):
            for mo, lo in enumerate(l_outs):
                d = li - lo + 1
                if 0 <= d < 3:
                    for co in range(cout):
                        W[ki * cin + ci, mo * cout + co] = w[co, ci, d]
    return W


def _band_parity(w, l_ins, l_out_base, half_l, cin, cout):
    """Banded matrix with parity-grouped output: cols = par*64 + lp*cout +
    co, l_out = l_out_base + 2*lp + par (even block cols 0:48, odd block
    cols 64:112; pads 48:64 and 112:128 are zeroed by the matmul so the
    full [128] tensor is initialized)."""
    K = len(l_ins) * cin
    W = np.zeros((K, 128), np.float32)
    for ki, li in enumerate(l_ins):
        for ci in range(cin):
            for par in range(2):
                for lp in range(half_l):
                    lo = l_out_base + 2 * lp + par
                    d = li - lo + 1
                    if 0 <= d < 3:
                        for co in range(cout):
                            W[ki * cin + ci, par * 64 + lp * cout + co] = w[co, ci, d]
    return W


def _pad48(W):
    """Insert 16 zero rows at row 48 (pooled tensors carry a pad block)."""
    return np.concatenate([W[:48], np.zeros((16,) + W.shape[1:], W.dtype), W[48:]], 0)


def _host_weights(w11, b11, w12, b12, w21, b21, w22, b22, Wl, bl):
    f32 = np.float32
    w11, w12, w21, w22, Wl = (np.asarray(a, f32) for a in (w11, w12, w21, w22, Wl))

    W11A = _band_first(w11, range(0, 24), range(0, 13), 1, 8)      # [24, 104]
    W11B = _band_first(w11, range(0, 24), range(11, 24), 1, 8)     # [24, 104]
    W12A = _band_parity(w12, range(0, 13), 0, 6, 8, 8)             # [104, 128]
    W12B = _band_parity(w12, range(11, 24), 12, 6, 8, 8)           # [104, 128]
    W21A = _pad48(_band_first(w21, range(0, 12), range(0, 7), 8, 16))   # [112, 112]
    W21B = _pad48(_band_first(w21, range(0, 12), range(5, 12), 8, 16))  # [112, 112]
    W22A = _band_parity(w22, range(0, 7), 0, 3, 16, 16)            # [112, 128]
    W22B = _band_parity(w22, range(5, 12), 6, 3, 16, 16)           # [112, 128]
    # torch flatten feature = c*6 + lp ; h4 row = lp*16 + c (plus pad48)
    WLIN = np.zeros((96, 24), f32)
    for lp in range(6):
        for c in range(16):
            WLIN[lp * 16 + c, :] = Wl[:, c * 6 + lp]
    WLIN = _pad48(WLIN)                                            # [112, 24]

    return {
        "w11a": W11A, "w11b": W11B, "w12a": W12A, "w12b": W12B,
        "w21a": W21A, "w21b": W21B, "w22a": W22A, "w22b": W22B,
        "wlin": WLIN,
        "b11v": np.tile(np.asarray(b11, f32), 13).reshape(104, 1),
        "b12v": np.tile(np.asarray(b12, f32), 16).reshape(128, 1),
        "b21v": np.tile(np.asarray(b21, f32), 7).reshape(112, 1),
        "b22v": np.tile(np.asarray(b22, f32), 8).reshape(128, 1),
        "blv": np.asarray(bl, f32).reshape(24, 1),
    }


# weight blob layout: (name, K, M) in packing order
_WSPEC = [
    ("w11a", 24, 104), ("w11b", 24, 104),
    ("w12a", 104, 128), ("w12b", 104, 128),
    ("w21a", 112, 112), ("w21b", 112, 112),
    ("w22a", 112, 128), ("w22b", 112, 128),
    ("wlin", 112, 24),
]
_WOFF = {}
_off = 0
for _n, _k, _m in _WSPEC:
    _WOFF[_n] = _off
    _off += _m
WBLOB_COLS = _off

_BSPEC = [("b11v", 104), ("b12v", 128), ("b21v", 112), ("b22v", 128), ("blv", 24)]
_BOFF = {n: i for i, (n, _) in enumerate(_BSPEC)}


def _pack_blobs(W, np_dt):
    wb = np.zeros((128, WBLOB_COLS), np_dt)
    for n, k, m in _WSPEC:
        assert W[n].shape == (k, m), (n, W[n].shape)
        wb[:k, _WOFF[n]:_WOFF[n] + m] = W[n].astype(np_dt)
    bb = np.zeros((128, len(_BSPEC)), np.float32)
    for n, p in _BSPEC:
        bb[:p, _BOFF[n]] = W[n][:, 0]
    return wb, bb


# ----------------------------------------------------- numpy device model
def emulate(x, np_dt=np.float16, **kw):
    """Pure-numpy emulation of the device dataflow (same banded matrices,
    same orderings, same cast points). Used to validate index math."""
    W = _host_weights(**kw)
    xt = np.ascontiguousarray(x.reshape(-1, L).T).astype(np_dt)  # [24, N]
    c = lambda a: a.astype(np_dt)

    def mm(wname, act):
        return c(W[wname]).astype(np.float32).T @ act.astype(np.float32)

    def relu_b(a, bias):
        return np.maximum(a + bias, 0.0)

    psA, psB = c(mm("w11a", xt)), c(mm("w11b", xt))
    h1a, h1b = c(relu_b(psA, W["b11v"])), c(relu_b(psB, W["b11v"]))
    psC, psD = c(mm("w12a", h1a)), c(mm("w12b", h1b))
    sA, sB = c(relu_b(psC, W["b12v"])), c(relu_b(psD, W["b12v"]))
    h2r = np.concatenate(
        [np.maximum(sA[0:64], sA[64:128]), np.maximum(sB[0:48], sB[64:112])], 0
    )
    psE, psF = c(mm("w21a", h2r)), c(mm("w21b", h2r))
    h3a, h3b = c(relu_b(psE, W["b21v"])), c(relu_b(psF, W["b21v"]))
    psG, psH = c(mm("w22a", h3a)), c(mm("w22b", h3b))
    sG, sH = c(relu_b(psG, W["b22v"])), c(relu_b(psH, W["b22v"]))
    h4r = np.concatenate(
        [np.maximum(sG[0:64], sG[64:128]), np.maximum(sH[0:48], sH[64:112])], 0
    )
    out = mm("wlin", h4r) + W["blv"]  # fp32
    return out.T.reshape(x.shape[0], x.shape[1], 24).astype(np.float32)


# --------------------------------------------------------- device builder
def build_kernel(n_samples, compute=COMPUTE, n_cores=N_CORES):
    cfg = _cfg(compute)
    DT, NT = cfg["dt"], cfg["nt"]
    MMC = cfg["mm_cast"]
    f32 = mybir.dt.float32
    n_tiles = n_samples // NT
    assert n_samples % NT == 0

    nc = bacc.Bacc(
        "TRN2",
        target_bir_lowering=False,
        debug=False,
        enable_asserts=False,
        num_devices=n_cores,
    )

    xt_d = nc.dram_tensor("xt", [n_tiles, 24, NT], DT, kind="ExternalInput").ap()
    wb_d = nc.dram_tensor("wblob", [128, WBLOB_COLS], DT, kind="ExternalInput").ap()
    bb_d = nc.dram_tensor("bblob", [128, len(_BSPEC)], f32,
                          kind="ExternalInput").ap()
    out_d = nc.dram_tensor("out", [n_tiles, 24, NT], f32, kind="ExternalOutput").ap()

    Relu = mybir.ActivationFunctionType.Relu
    Add, Max = mybir.AluOpType.add, mybir.AluOpType.max

    def mmop(ap):
        return ap.bitcast(MMC) if MMC is not None else ap

    # matmul fp32 PSUM output must stay inside one 2KB bank -> <=512 cols
    MMN = min(NT, 512)

    with tile.TileContext(nc) as tc:
        with (
            tc.tile_pool(name="consts", bufs=1) as cpool,
            tc.tile_pool(name="xin", bufs=6) as xpool,
            tc.tile_pool(name="acts", bufs=4) as apool,
            tc.tile_pool(name="outs", bufs=3) as opool,
            tc.tile_pool(name="ps", bufs=3, space="PSUM") as pspool,
            tc.tile_pool(name="pslin", bufs=1, space="PSUM") as lpool,
        ):
            # prefetch the ACT spline-table set (~2.7us) during the blob
            # DMAs: a dummy ACTIVATE forces walrus to place the table load
            # at the head of ACT's stream instead of before tile 0's evac
            warm = cpool.tile([1, 2], f32, tag="actwarm")
            nc.vector.memset(warm[:], 0.0)
            nc.scalar.activation(warm[:], warm[:], Relu, bias=0.0)

            wsb = cpool.tile([128, WBLOB_COLS], DT, tag="wblob")
            bsb = cpool.tile([128, len(_BSPEC)], f32, tag="bblob")
            nc.sync.dma_start(wsb[:], wb_d)
            nc.sync.dma_start(bsb[:], bb_d)

            def w(name):
                k, m = next((kk, mm_) for nn, kk, mm_ in _WSPEC if nn == name)
                return mmop(wsb[0:k, _WOFF[name]:_WOFF[name] + m])

            def bias(name):
                p = next(pp for nn, pp in _BSPEC if nn == name)
                return bsb[0:p, _BOFF[name]:_BOFF[name] + 1]

            def mm(out_ps, wname, rhs_sb):
                for j in range(0, NT, MMN):
                    nc.tensor.matmul(out_ps[:, j:j + MMN], w(wname),
                                     mmop(rhs_sb[:, j:j + MMN]),
                                     start=True, stop=True)

            # ---- software-pipelined emission -------------------------
            # Engines execute their instruction streams IN ORDER, so a
            # depth-first per-tile emission serializes tiles (the PE sits
            # behind its own next-layer matmuls, which wait on the current
            # tile's evacuations). Emitting the five stages SKEWED across
            # tiles interleaves independent work in every engine's queue.
            h1 = {}
            h2 = {}
            h3 = {}
            h4 = {}

            def s1_conv11(t):
                xt_t = xpool.tile([24, NT], DT, tag="xt")
                nc.sync.dma_start(xt_t[:], xt_d[t])
                psA = pspool.tile([104, NT], f32, tag="ps")
                psB = pspool.tile([104, NT], f32, tag="ps")
                mm(psA, "w11a", xt_t)
                mm(psB, "w11b", xt_t)
                h1a = apool.tile([104, NT], DT, tag="h1a")
                h1b = apool.tile([104, NT], DT, tag="h1b")
                nc.scalar.activation(h1a[:], psA[:], Relu, bias=bias("b11v"))
                nc.scalar.activation(h1b[:], psB[:], Relu, bias=bias("b11v"))
                h1[t] = (h1a, h1b)

            def s2_conv12(t):
                h1a, h1b = h1.pop(t)
                psC = pspool.tile([128, NT], f32, tag="ps")
                psD = pspool.tile([128, NT], f32, tag="ps")
                mm(psC, "w12a", h1a)
                mm(psD, "w12b", h1b)
                s12a = apool.tile([128, NT], DT, tag="s12a")
                s12b = apool.tile([128, NT], DT, tag="s12b")
                nc.scalar.activation(s12a[:], psC[:], Relu, bias=bias("b12v"))
                nc.scalar.activation(s12b[:], psD[:], Relu, bias=bias("b12v"))
                mv1 = apool.tile([64, NT], DT, tag="mv1")
                mv2 = apool.tile([112, NT], DT, tag="mv2")
                nc.scalar.dma_start(mv1[0:64, :], s12a[64:128, :])
                nc.scalar.dma_start(mv2[64:112, :], s12b[0:48, :])
                h2r = apool.tile([112, NT], DT, tag="h2r")
                nc.vector.tensor_max(h2r[0:64, :], s12a[0:64, :], mv1[0:64, :])
                nc.vector.tensor_max(h2r[64:112, :], s12b[64:112, :],
                                     mv2[64:112, :])
                h2[t] = h2r

            def s3_conv21(t):
                h2r = h2.pop(t)
                psE = pspool.tile([112, NT], f32, tag="ps")
                psF = pspool.tile([112, NT], f32, tag="ps")
                mm(psE, "w21a", h2r)
                mm(psF, "w21b", h2r)
                h3a = apool.tile([112, NT], DT, tag="h3a")
                h3b = apool.tile([112, NT], DT, tag="h3b")
                nc.vector.tensor_scalar(h3a[:], psE[:], bias("b21v"), 0.0,
                                        Add, Max)
                nc.vector.tensor_scalar(h3b[:], psF[:], bias("b21v"), 0.0,
                                        Add, Max)
                h3[t] = (h3a, h3b)

            def s4_conv22(t):
                h3a, h3b = h3.pop(t)
                psG = pspool.tile([128, NT], f32, tag="ps")
                psH = pspool.tile([128, NT], f32, tag="ps")
                mm(psG, "w22a", h3a)
                mm(psH, "w22b", h3b)
                s22a = apool.tile([128, NT], DT, tag="s22a")
                s22b = apool.tile([128, NT], DT, tag="s22b")
                nc.scalar.activation(s22a[:], psG[:], Relu, bias=bias("b22v"))
                nc.scalar.activation(s22b[:], psH[:], Relu, bias=bias("b22v"))
                mv3 = apool.tile([64, NT], DT, tag="mv3")
                mv4 = apool.tile([112, NT], DT, tag="mv4")
                nc.scalar.dma_start(mv3[0:64, :], s22a[64:128, :])
                nc.scalar.dma_start(mv4[64:112, :], s22b[0:48, :])
                h4r = apool.tile([112, NT], DT, tag="h4r")
                nc.vector.tensor_max(h4r[0:64, :], s22a[0:64, :], mv3[0:64, :])
                nc.vector.tensor_max(h4r[64:112, :], s22b[64:112, :],
                                     mv4[64:112, :])
                h4[t] = h4r

            def s5_linear(t):
                h4r = h4.pop(t)
                psI = lpool.tile([24, NT], f32, tag="pslin")
                mm(psI, "wlin", h4r)
                osb = opool.tile([24, NT], f32, tag="osb")
                nc.vector.tensor_scalar_add(osb[:], psI[:], bias("blv"))
                nc.gpsimd.dma_start(out_d[t], osb[:])

            stages = [s1_conv11, s2_conv12, s3_conv21, s4_conv22, s5_linear]
            for step in range(n_tiles + len(stages) - 1):
                for s, fn in enumerate(stages):
                    t = step - s
                    if 0 <= t < n_tiles:
                        fn(t)

    nc.compile()
    return nc


# ------------------------------------------------------------- entry point
def _prep_in_maps(x, weights, compute=COMPUTE):
    cfg = _cfg(compute)
    np_dt = cfg["np_dt"]
    nt = cfg["nt"]
    W = _host_weights(**weights)
    wb, bb = _pack_blobs(W, np_dt)
    xt = np.ascontiguousarray(x.reshape(SB, L).T).astype(np_dt)  # [24, SB]
    in_maps = []
    for c in range(N_CORES):
        xc = xt[:, c * CORE_N:(c + 1) * CORE_N]  # [24, CORE_N]
        in_maps.append({
            "xt": np.ascontiguousarray(
                xc.reshape(24, CORE_N // nt, nt).transpose(1, 0, 2)
            ),
            "wblob": wb,
            "bblob": bb,
        })
    return in_maps


def kernel(x, w11, b11, w12, b12, w21, b21, w22, b22, Wl, bl):
    weights = dict(w11=w11, b11=b11, w12=w12, b12=b12, w21=w21, b21=b21,
                   w22=w22, b22=b22, Wl=Wl, bl=bl)
    x = np.asarray(x, np.float32)
    nc = build_kernel(CORE_N, COMPUTE)
    in_maps = _prep_in_maps(x, weights, COMPUTE)
    res = run_bass_kernel_spmd(nc, in_maps, list(range(N_CORES)))
    outs = [
        res.results[c]["out"].transpose(1, 0, 2).reshape(24, CORE_N)
        for c in range(N_CORES)
    ]
    full = np.concatenate(outs, axis=1)  # [24, SB]
    return np.ascontiguousarray(full.T).reshape(S, B, 24).astype(np.float32)



# revision 14
# speedup vs baseline: 1.0616x; 1.0616x over previous
"""Trainium2 Bass kernel for nn_CNN_pre_LSTM (dense_cnn), v2.

Reference computation per sample (L=24):
    h = relu(conv1d(x, w11, b11))    # 1 -> 8 ch, k=3, same pad
    h = relu(conv1d(h, w12, b12))    # 8 -> 8
    h = maxpool2(h)                  # L 24 -> 12
    h = relu(conv1d(h, w21, b21))    # 8 -> 16
    h = relu(conv1d(h, w22, b22))    # 16 -> 16
    h = maxpool2(h)                  # L 12 -> 6
    y = h.reshape(96) @ Wl.T + bl    # 96 -> 24

Pure data parallel over the fused (S*B) batch across 8 cores; 16384
samples per core, processed as 16 tiles of NT=1024. Activations live as
[feature, batch] (features on SBUF partitions); each conv layer is one
dense banded matmul per <=128-col output block (host-built matrices,
l-major/c-minor feature order, halo-overlapped l-halves; no PSUM
accumulation anywhere).

Two hardware constraints shape all layouts (both verified the hard way):
  * compute-engine operands (ACT/DVE/Pool/PE) must live on identical
    partition ranges AND every compute AP's base partition must be 0,
    32, or 64 -- only DMA can move data across partitions or touch
    other bases. Hence the parity-pooling pad blocks: pooled tensors
    are [48 real | 16 pad | 48 real] so both maxes write at legal
    bases, and the consumer matmul's weights carry zero rows at the
    pads.
  * matmul fp32 PSUM output must stay inside one 2KB bank (<=512 cols).

v2 structure (vs the 18-matmul/tile v1 at 170.4us):

  * The final Linear (96->24) is MERGED into conv11-A's matmul as a
    block-diagonal stationary [120 x 128]: rows 0:96 x cols 104:128 =
    WLIN over compact pooled h4, rows 96:120 x cols 0:104 = W11A over
    x. The moving operand is one tile h4x = [h4-A 0:48 | h4-B 48:96 |
    x 96:120] pairing tile t's conv11 with tile t-P's linear (P=6), so
    the maxpool chain latency (~6.5us: evac, align DMA, max, move DMA)
    sits far off the critical path. 16 matmul instrs per tile + 2P
    drain instrs instead of 18/tile.
  * h4 is compacted to 96 rows: the A-half max writes h4x[0:48]
    directly; the B-half max runs IN-PLACE at s22b[64:112] and a DMA
    moves it to h4x[48:96] (DMA is the only base-free engine).
  * conv11B reads x via rhs = h4x[64:120] (legal base 64) against a
    stationary whose rows 0:32 are zeros (the h4-B rows it overlaps).
  * Engine balance per tile (PE is the only engine near its roofline):
    PE 8 matmul pairs (~7.0us), ACT 6 relu evacuations (~6.6us), DVE
    [h3a, out ts_add, h3b, 4 pool-maxes] (~6.4us). walrus rejects
    TensorTensor on Pool (engine ISA check), so gpsimd only does
    memsets. PSUM = one 4-buffer ring of [128,1024] fp32 tiles (8
    banks exactly); slot-reuse distances are 4+ matmul pairs, ~2.5x
    the evacuation latency.
  * The out columns ride at psM[104:128]; their ts_add evacuation runs
    on [64:128] (legal base; rows 64:104 are h1a columns, harmlessly
    re-evacuated into unused osb rows) and the store DMA reads
    osb[104:128]. Output DMAs go on the SP HWDGE queue (TRN2 HWDGE =
    SP+ACT only; v1 burned ~0.65us/tile of gpsimd SWDGE dispatch).
  * All weights ship as one packed fp16 blob, split so the merged
    stationary + W11B land first and LDWEIGHTS can start ~1.5us
    earlier; biases as one fp32 blob whose per-row layouts match each
    evacuation's partition range (out bias at rows 104:128 etc).

The PE on this part runs at a fixed 1.2 GHz (the HAM clock gate never
opens even under 6us+ of continuous matmuls), so matmul cost is
N/1.2GHz per instruction: 512-col chunks = 427ns; 16x16+2P=268 instrs
=> ~114.3us PE floor. fp8 (DoubleRow, 2x) was measured numerically and
fails the 2e-2 gate (>=2.6e-2 even for a single layer in e4m3).

DRAM partition strides must stay <= 32KB (64KB strides crash the
device): input [n_tiles, 24, NT] fp16, output [n_tiles, 24, NT] fp32.
"""

import numpy as np

import concourse.bass as bass
import concourse.tile as tile
import concourse.mybir as mybir
from concourse import bacc
from concourse.bass_utils import run_bass_kernel_spmd

# ---------------------------------------------------------------- config
N_CORES = 8
S, B, L = 512, 256, 24
SB = S * B
CORE_N = SB // N_CORES  # 16384

# linear(t-P) rides in conv11(t)'s matmul; P tiles of slack for the
# maxpool chain (evac -> align DMA -> max -> move DMA) to complete.
P_BACK = 6

# compute dtype for matmul operands / intermediate activations:
#   "fp16"  : float16 operands, fp32 PSUM accumulate, NT=1024
#   "fp32"  : exact fp32 (PE 4x slower), NT=512  (debug only)
COMPUTE = "fp16"


def _cfg(compute):
    if compute == "fp16":
        return dict(dt=mybir.dt.float16, np_dt=np.float16, nt=1024)
    if compute == "fp32":
        return dict(dt=mybir.dt.float32, np_dt=np.float32, nt=512)
    raise ValueError(compute)


# ------------------------------------------------- host weight transforms
#
# Feature row orderings (all l-major, c-minor):
#   x:    rows l in [0,24)                              -> 24 rows
#   h1 block A: rows (l, c)  l in [0,13), c in [0,8)    -> 104 rows
#   h1 block B: rows (l, c)  l in [11,24)               -> 104 rows
#   conv12 out (parity): rows par*64 + lp*8 + c         -> 112 rows used
#   pooled h2:  rows [lp 0..5 x8ch | 16 pad | lp 6..11 x8ch] = 112
#   h3 block A: rows (l, c16) l in [0,7)                -> 112 rows
#   h3 block B: rows (l-5, c16) l in [5,12)             -> 112 rows
#   conv22A out (parity): rows par*64 + lp*16 + c       -> 112 rows used
#   conv22B out: odd l at lp*16+c in 0:48, even l at 64+lp*16+c
#   pooled h4 (compact): rows lp*16 + c, lp in [0,6)    -> 96 rows
#   merged moving tile h4x: [h4 0:96 | x 96:120]
#   merged out cols: 0:104 = h1a(t), 104:128 = y(t-P)

def _band_first(w, l_ins, l_outs, cin, cout):
    """Dense banded matrix [len(l_ins)*cin, len(l_outs)*cout] for a k=3
    'same' conv, rows (l_in, ci) l-major, cols (l_out, co) l-major."""
    K = len(l_ins) * cin
    M = len(l_outs) * cout
    W = np.zeros((K, M), np.float32)
    for ki, li in enumerate(l_ins):
        for ci in range(cin):
            for mo, lo in enumerate(l_outs):
                d = li - lo + 1
                if 0 <= d < 3:
                    for co in range(cout):
                        W[ki * cin + ci, mo * cout + co] = w[co, ci, d]
    return W


def _band_parity(w, l_ins, l_out_base, half_l, cin, cout):
    """Banded matrix with parity-grouped output: cols = par*64 + lp*cout +
    co, l_out = l_out_base + 2*lp + par (even block cols 0:48, odd block
    cols 64:112; pads 48:64 and 112:128 are zeroed by the matmul so the
    full [128] tensor is initialized)."""
    K = len(l_ins) * cin
    W = np.zeros((K, 128), np.float32)
    for ki, li in enumerate(l_ins):
        for ci in range(cin):
            for par in range(2):
                for lp in range(half_l):
                    lo = l_out_base + 2 * lp + par
                    d = li - lo + 1
                    if 0 <= d < 3:
                        for co in range(cout):
                            W[ki * cin + ci, par * 64 + lp * cout + co] = w[co, ci, d]
    return W


def _pad48(W):
    """Insert 16 zero rows at row 48 (pooled tensors carry a pad block)."""
    return np.concatenate([W[:48], np.zeros((16,) + W.shape[1:], W.dtype), W[48:]], 0)


def _host_weights(w11, b11, w12, b12, w21, b21, w22, b22, Wl, bl):
    f32 = np.float32
    w11, w12, w21, w22, Wl = (np.asarray(a, f32) for a in (w11, w12, w21, w22, Wl))
    b11, b12, b21, b22, bl = (np.asarray(a, f32) for a in (b11, b12, b21, b22, bl))

    W11A = _band_first(w11, range(24), range(0, 13), 1, 8)       # [24, 104]
    W11B = _band_first(w11, range(24), range(11, 24), 1, 8)      # [24, 104]
    # conv11B rhs is h4x[64:120]; rows 0:32 overlap h4-B and are zeroed
    W11BX = np.concatenate([np.zeros((32, 104), f32), W11B], 0)  # [56, 104]

    W12A = _band_parity(w12, range(0, 13), 0, 6, 8, 8)           # [104, 128]
    W12B = _band_parity(w12, range(11, 24), 12, 6, 8, 8)         # [104, 128]
    W21A = _pad48(_band_first(w21, range(0, 12), range(0, 7), 8, 16))   # [112, 112]
    W21B = _pad48(_band_first(w21, range(0, 12), range(5, 12), 8, 16))  # [112, 112]
    W22A = _band_parity(w22, range(0, 7), 0, 3, 16, 16)          # [112, 128]
    # conv22B: odd l_out at cols 0:48, even at 64:112 (pad 48:64)
    W22B = np.zeros((112, 112), f32)
    F22B = _band_first(w22, range(5, 12), range(6, 12), 16, 16)  # [112, 96]
    for i, lo in enumerate(range(6, 12)):
        base = ((lo - 7) // 2) * 16 if lo % 2 else 64 + ((lo - 6) // 2) * 16
        W22B[:, base:base + 16] = F22B[:, i * 16:(i + 1) * 16]

    # torch flatten feature = c*6 + lp ; compact pooled h4 row = lp*16+c
    WLIN = np.zeros((96, 24), f32)
    for lp in range(6):
        for c in range(16):
            WLIN[lp * 16 + c, :] = Wl[:, c * 6 + lp]

    WMG = np.zeros((120, 128), f32)
    WMG[0:96, 104:128] = WLIN
    WMG[96:120, 0:104] = W11A

    def bvec(pieces):
        v = np.zeros(128, f32)
        for r0, vals in pieces:
            v[r0:r0 + len(vals)] = vals
        return v

    t = np.tile
    return {
        "wmg": WMG, "w11b": W11BX,
        "w12a": W12A, "w12b": W12B,
        "w21a": W21A, "w21b": W21B,
        "w22a": W22A, "w22b": W22B,
        "b11v": bvec([(0, t(b11, 13))]),
        "b12v": bvec([(0, t(b12, 16))]),
        "b21v": bvec([(0, t(b21, 7))]),
        "b22a": bvec([(0, t(b22, 8))]),
        "b22b": bvec([(0, t(b22, 3)), (64, t(b22, 3))]),
        "blv": bvec([(104, bl)]),
    }


# weight blob layout: (name, row0, K, M) in packing order. wmg + w11b
# form the first split (needed by tile 0's first matmuls).
_WSPEC = [
    ("wmg", 0, 120, 128), ("w11b", 64, 56, 104),
    ("w12a", 0, 104, 128), ("w12b", 0, 104, 128),
    ("w21a", 0, 112, 112), ("w21b", 0, 112, 112),
    ("w22a", 0, 112, 128), ("w22b", 0, 112, 112),
]
_WOFF = {}
_off = 0
for _n, _r0, _k, _m in _WSPEC:
    _WOFF[_n] = _off
    _off += _m
WBLOB_COLS = _off           # 952
WBLOB_FIRST = 128 + 104     # wmg + w11b

_BSPEC = ["b11v", "b12v", "b21v", "b22a", "b22b", "blv"]
_BOFF = {n: i for i, n in enumerate(_BSPEC)}


def _pack_blobs(W, np_dt):
    wb = np.zeros((128, WBLOB_COLS), np_dt)
    for n, r0, k, m in _WSPEC:
        assert W[n].shape == (k, m), (n, W[n].shape)
        wb[r0:r0 + k, _WOFF[n]:_WOFF[n] + m] = W[n].astype(np_dt)
    bb = np.zeros((128, len(_BSPEC)), np.float32)
    for n in _BSPEC:
        bb[:, _BOFF[n]] = W[n]
    return wb, bb


# ----------------------------------------------------- numpy device model
def emulate(x, np_dt=np.float16, **kw):
    """Pure-numpy emulation of the device dataflow (same banded matrices,
    same orderings, same cast points). Used to validate index math."""
    W = _host_weights(**kw)
    xt = np.ascontiguousarray(x.reshape(-1, L).T).astype(np_dt)  # [24, N]
    c = lambda a: a.astype(np_dt)
    f32 = np.float32

    def mm(wname, act, rows):
        return (c(W[wname]).astype(f32).T @ act.astype(f32))[:rows]

    def relu_b(a, bname):
        b = W[bname][:a.shape[0], None]
        return np.maximum(a + b, 0.0)

    h1a = c(relu_b(mm("wmg", np.concatenate(
        [np.zeros((96, xt.shape[1]), np_dt), xt]), 104), "b11v"))
    h1b = c(relu_b(mm("w11b", np.concatenate(
        [np.zeros((32, xt.shape[1]), np_dt), xt]), 104), "b11v"))
    s12a = c(relu_b(mm("w12a", h1a, 128), "b12v"))
    s12b = c(relu_b(mm("w12b", h1b, 128), "b12v"))
    h2r = np.concatenate(
        [np.maximum(s12a[0:64], s12a[64:128]),
         np.maximum(s12b[64:112], s12b[0:48])], 0)
    h3a = c(relu_b(mm("w21a", h2r, 112), "b21v"))
    h3b = c(relu_b(mm("w21b", h2r, 112), "b21v"))
    s22a = c(relu_b(mm("w22a", h3a, 128), "b22a"))
    s22b = c(relu_b(mm("w22b", h3b, 112), "b22b"))
    h4 = np.concatenate(
        [np.maximum(s22a[0:48], s22a[64:112]),
         np.maximum(s22b[64:112], s22b[0:48])], 0)
    h4x = np.concatenate([h4, xt], 0)           # [120, N]
    out = mm("wmg", h4x, 128)[104:128] + W["blv"][104:128, None]
    return out.T.reshape(x.shape[0], x.shape[1], 24).astype(f32)


# --------------------------------------------------------- device builder
def build_kernel(n_samples, compute=COMPUTE, n_cores=N_CORES):
    cfg = _cfg(compute)
    DT, NT = cfg["dt"], cfg["nt"]
    f32 = mybir.dt.float32
    n_tiles = n_samples // NT
    assert n_samples % NT == 0
    P = P_BACK
    last = n_tiles - 1

    nc = bacc.Bacc(
        "TRN2",
        target_bir_lowering=False,
        debug=False,
        enable_asserts=False,
        num_devices=n_cores,
    )

    xt_d = nc.dram_tensor("xt", [n_tiles, 24, NT], DT, kind="ExternalInput").ap()
    wb_d = nc.dram_tensor("wblob", [128, WBLOB_COLS], DT, kind="ExternalInput").ap()
    bb_d = nc.dram_tensor("bblob", [128, len(_BSPEC)], f32,
                          kind="ExternalInput").ap()
    out_d = nc.dram_tensor("out", [n_tiles, 24, NT], f32, kind="ExternalOutput").ap()

    Relu = mybir.ActivationFunctionType.Relu
    Add, Max = mybir.AluOpType.add, mybir.AluOpType.max

    MMN = min(NT, 512)  # matmul fp32 PSUM output must stay inside one bank

    with tile.TileContext(nc) as tc:
        with (
            tc.tile_pool(name="consts", bufs=1) as cpool,
            tc.tile_pool(name="h4x", bufs=P + 1) as xpool,
            tc.tile_pool(name="acts", bufs=3) as apool,
            tc.tile_pool(name="outs", bufs=3) as opool,
            tc.tile_pool(name="ps", bufs=4, space="PSUM") as pspool,
        ):
            # prefetch the ACT spline-table set (~1.3us) during the blob
            # DMAs: a dummy ACTIVATE forces walrus to place the table load
            # at the head of ACT's stream instead of before tile 0's evac
            warm = cpool.tile([1, 2], f32, tag="actwarm")
            nc.vector.memset(warm[:], 0.0)
            nc.scalar.activation(warm[:], warm[:], Relu, bias=0.0)

            wsb = cpool.tile([128, WBLOB_COLS], DT, tag="wblob")
            bsb = cpool.tile([128, len(_BSPEC)], f32, tag="bblob")
            # first-needed weights (merged + W11B) in their own small DMA
            nc.sync.dma_start(wsb[:, 0:WBLOB_FIRST], wb_d[:, 0:WBLOB_FIRST])
            nc.sync.dma_start(bsb[:], bb_d)
            nc.sync.dma_start(wsb[:, WBLOB_FIRST:], wb_d[:, WBLOB_FIRST:])

            def w(name):
                r0, k, m = next((r, kk, mm_) for nn, r, kk, mm_ in _WSPEC
                                if nn == name)
                return wsb[r0:r0 + k, _WOFF[name]:_WOFF[name] + m]

            def bias(name, lo, hi):
                return bsb[lo:hi, _BOFF[name]:_BOFF[name] + 1]

            _WM = {n: m for n, _r, _k, m in _WSPEC}
            _WR = {n: r for n, r, _k, _m in _WSPEC}

            def mm(ps_ap, wname, rhs_ap):
                m = _WM[wname]
                for j in range(0, NT, MMN):
                    nc.tensor.matmul(ps_ap[0:m, j:j + MMN], w(wname),
                                     rhs_ap[:, j:j + MMN],
                                     start=True, stop=True)

            # h4x tiles: rows 0:96 = compact pooled h4 (written by tile
            # t-P's maxes / move-DMA), rows 96:120 = x(t) (prefetch).
            # Tiles t < P read zeros (their merged out columns are
            # discarded); drain tiles read stale x rows (zero weights).
            h4x = {}

            def h4x_tile(t):
                if t not in h4x:
                    h4x[t] = xpool.tile([120, NT], DT, tag="h4x",
                                        name=f"h4x{t}")
                return h4x[t]

            for t in range(min(P, n_tiles + P)):
                nc.gpsimd.memset(h4x_tile(t)[:], 0.0)
            nc.sync.dma_start(h4x_tile(0)[96:120, :], xt_d[0])
            if n_tiles > 1:
                nc.sync.dma_start(h4x_tile(1)[96:120, :], xt_d[1])

            psM = {}
            psEF = {}
            h1 = {}
            s12 = {}
            mv12 = {}
            h2 = {}
            h3 = {}
            s22 = {}
            mv22 = {}
            osb = {}

            for k in range(n_tiles + P):
                # ---- s3 evac (1st half) of tile k-3 ------------------
                t = k - 3
                if 0 <= t <= last:
                    psE, psF = psEF[t]
                    h3a = apool.tile([112, NT], DT, tag="h3a")
                    nc.vector.tensor_scalar(h3a[:], psE[0:112, :],
                                            bias("b21v", 0, 112), 0.0, Add, Max)
                    h3[t] = (h3a, None)

                # ---- s1(k): merged (h1a(k) + out(k-P)) ---------------
                pm = pspool.tile([128, NT], f32, tag="ps")
                mm(pm, "wmg", h4x_tile(k)[0:120, :])
                psM[k] = pm
                if P <= k:
                    ot = opool.tile([128, NT], f32, tag="osb")
                    nc.vector.tensor_scalar_add(ot[64:128, :],
                                                pm[64:128, :],
                                                bias("blv", 64, 128))
                    osb[k] = ot

                # ---- s3 evac (2nd half; keeps DVE order) -------------
                t = k - 3
                if 0 <= t <= last:
                    psE, psF = psEF.pop(t)
                    h3a, _ = h3[t]
                    h3b = apool.tile([112, NT], DT, tag="h3b")
                    nc.vector.tensor_scalar(h3b[:], psF[0:112, :],
                                            bias("b21v", 0, 112), 0.0, Add, Max)
                    h3[t] = (h3a, h3b)

                # ---- s1(k): conv11B ----------------------------------
                if k <= last:
                    pb = pspool.tile([128, NT], f32, tag="ps")
                    mm(pb, "w11b", h4x_tile(k)[64:120, :])
                    h1a = apool.tile([104, NT], DT, tag="h1a")
                    h1b = apool.tile([104, NT], DT, tag="h1b")
                    nc.scalar.activation(h1a[:], psM[k][0:104, :], Relu,
                                         bias=bias("b11v", 0, 104))
                    nc.scalar.activation(h1b[:], pb[0:104, :], Relu,
                                         bias=bias("b11v", 0, 104))
                    h1[k] = (h1a, h1b)

                # ---- maxes of tile k-4's conv22 -> h4x(k+2) ----------
                # A-half maxes straight into h4x[0:48]; B-half maxes
                # in-place at s22b[64:112] and the SP move-DMA lands it
                # at h4x[48:96] (base-48 writes are DMA-only).
                t = k - 4
                if 0 <= t <= last:
                    s22a, s22b = s22.pop(t)
                    mv = mv22.pop(t)
                    tgt = h4x_tile(t + P)
                    if t + P > last:
                        # drain tile: x rows never prefetched; zero them
                        # (their weight rows are zero; keeps all reads
                        # initialized). memset is a compute op: base 64.
                        nc.gpsimd.memset(tgt[64:120, :], 0.0)
                    # walrus rejects TensorTensor on Pool (engine check);
                    # all maxes run on DVE (fp16 SBUF = 2x mode, ~0.66us)
                    nc.vector.tensor_max(tgt[0:48, :], s22a[0:48, :],
                                         mv[0:48, :])
                    nc.vector.tensor_max(s22b[64:112, :], s22b[64:112, :],
                                         mv[64:112, :])
                    nc.sync.dma_start(tgt[48:96, :], s22b[64:112, :])

                # ---- input prefetch ----------------------------------
                if k + 2 <= last:
                    nc.sync.dma_start(h4x_tile(k + 2)[96:120, :], xt_d[k + 2])

                # ---- maxes of tile k-2's conv12 -> h2r(k-2) ----------
                t = k - 2
                if 0 <= t <= last:
                    s12a, s12b = s12.pop(t)
                    mv = mv12.pop(t)
                    h2r = apool.tile([112, NT], DT, tag="h2r")
                    nc.vector.tensor_max(h2r[0:64, :], s12a[0:64, :],
                                         mv[0:64, :])
                    nc.vector.tensor_max(h2r[64:112, :], s12b[64:112, :],
                                         mv[64:112, :])
                    h2[t] = h2r

                # ---- s2(k-1): conv12 ---------------------------------
                t = k - 1
                if 0 <= t <= last:
                    h1a, h1b = h1.pop(t)
                    psC = pspool.tile([128, NT], f32, tag="ps")
                    psD = pspool.tile([128, NT], f32, tag="ps")
                    mm(psC, "w12a", h1a[:])
                    mm(psD, "w12b", h1b[:])
                    s12a = apool.tile([128, NT], DT, tag="s12a")
                    s12b = apool.tile([128, NT], DT, tag="s12b")
                    nc.scalar.activation(s12a[:], psC[:], Relu,
                                         bias=bias("b12v", 0, 128))
                    nc.scalar.activation(s12b[:], psD[:], Relu,
                                         bias=bias("b12v", 0, 128))
                    mv = apool.tile([112, NT], DT, tag="mv12")
                    nc.scalar.dma_start(mv[0:64, :], s12a[64:128, :])
                    nc.scalar.dma_start(mv[64:112, :], s12b[0:48, :])
                    s12[t] = (s12a, s12b)
                    mv12[t] = mv

                # ---- s4(k-3): conv22 ---------------------------------
                t = k - 3
                if 0 <= t <= last:
                    h3a, h3b = h3.pop(t)
                    psG = pspool.tile([128, NT], f32, tag="ps")
                    psH = pspool.tile([128, NT], f32, tag="ps")
                    mm(psG, "w22a", h3a[:])
                    mm(psH, "w22b", h3b[:])
                    s22a = apool.tile([128, NT], DT, tag="s22a")
                    s22b = apool.tile([112, NT], DT, tag="s22b")
                    nc.scalar.activation(s22a[:], psG[:], Relu,
                                         bias=bias("b22a", 0, 128))
                    nc.scalar.activation(s22b[:], psH[0:112, :], Relu,
                                         bias=bias("b22b", 0, 112))
                    mv = apool.tile([112, NT], DT, tag="mv22")
                    nc.scalar.dma_start(mv[0:48, :], s22a[64:112, :])
                    # DVE has no HWDGE on TRN2; SP issues the DVE-produced
                    # block's align
                    nc.sync.dma_start(mv[64:112, :], s22b[0:48, :])
                    s22[t] = (s22a, s22b)
                    mv22[t] = mv

                # ---- s3(k-2): conv21 matmuls (evacs next step) -------
                t = k - 2
                if 0 <= t <= last:
                    h2r = h2.pop(t)
                    psE = pspool.tile([128, NT], f32, tag="ps")
                    psF = pspool.tile([128, NT], f32, tag="ps")
                    mm(psE, "w21a", h2r[:])
                    mm(psF, "w21b", h2r[:])
                    psEF[t] = (psE, psF)

                # ---- output store ------------------------------------
                if P <= k:
                    nc.sync.dma_start(out_d[k - P], osb.pop(k)[104:128, :])
                    del psM[k]

    nc.compile()
    return nc


# ------------------------------------------------------------- entry point
def _prep_in_maps(x, weights, compute=COMPUTE):
    cfg = _cfg(compute)
    np_dt = cfg["np_dt"]
    nt = cfg["nt"]
    W = _host_weights(**weights)
    wb, bb = _pack_blobs(W, np_dt)
    xt = np.ascontiguousarray(x.reshape(SB, L).T).astype(np_dt)  # [24, SB]
    in_maps = []
    for c in range(N_CORES):
        xc = xt[:, c * CORE_N:(c + 1) * CORE_N]  # [24, CORE_N]
        in_maps.append({
            "xt": np.ascontiguousarray(
                xc.reshape(24, CORE_N // nt, nt).transpose(1, 0, 2)
            ),
            "wblob": wb,
            "bblob": bb,
        })
    return in_maps


def kernel(x, w11, b11, w12, b12, w21, b21, w22, b22, Wl, bl):
    weights = dict(w11=w11, b11=b11, w12=w12, b12=b12, w21=w21, b21=b21,
                   w22=w22, b22=b22, Wl=Wl, bl=bl)
    x = np.asarray(x, np.float32)
    nc = build_kernel(CORE_N, COMPUTE)
    in_maps = _prep_in_maps(x, weights, COMPUTE)
    res = run_bass_kernel_spmd(nc, in_maps, list(range(N_CORES)))
    outs = [
        res.results[c]["out"].transpose(1, 0, 2).reshape(24, CORE_N)
        for c in range(N_CORES)
    ]
    full = np.concatenate(outs, axis=1)  # [24, SB]
    return np.ascontiguousarray(full.T).reshape(S, B, 24).astype(np.float32)
